# revision 23
# baseline (speedup 1.0000x reference)
"""Trainium2 Bass kernel for nn_Aggregation (SAN-style local aggregation).

out[n, g*32+cc, h, w] = sum_{kh,kw} input[n, g*32+cc, h-3+kh, w-3+kw] * weight[n, cc, kh*7+kw, h, w]

Sharding: data-parallel over batch N=16 across 8 NeuronCores (2 images/core).

Per-core layout:
  partition p = cc*4 + blk   (cc in [0,32): weight channel, blk in [0,4): block of 8 output rows)
  in_pad[p][n, g, r, col] = zero-padded input rows [blk*8-3, blk*8+11), cols [-3, 35)
  w_t[p][n, kk, hb, w]    = weight[n, cc, kk, blk*8+hb, w]
  For each tap kk=(kh,kw): acc[p][n,g,hb,w] += in_pad[p][n,g,hb+kh,w+kw] * w_t[p][n,kk,hb,w]
  (weight broadcast over g via stride-0 access pattern)

Mode "fp16row" (default): products and within-row (7-tap) accumulation in fp16
on the DVE at 2x rate; row sums flushed into an fp32 accumulator. A second
input copy shifted by one column keeps odd-kw taps 4B-aligned so the DVE's
2x perf mode stays engaged. Max abs error vs fp32 reference ~7e-4 of absmax.
Mode "fp32": everything fp32 (exact, ~2x slower).
"""

import numpy as np

N, C, H, W = 16, 256, 32, 32
K, PAD = 7, 3
CC, G = 32, 8
KK = K * K
NCORES = 8
NPC = N // NCORES
BLK, HB = 4, 8
R, COLP = HB + 2 * PAD, W + 2 * PAD  # 14, 38

MODE = "fp16row"
WCHUNK = False

_cache = {}


def _build(mode):
    import concourse.bacc as bacc
    import concourse.mybir as mybir
    import concourse.tile as tile

    fp32 = mybir.dt.float32
    fp16 = mybir.dt.float16
    cdt = fp32 if mode == "fp32" else fp16  # compute dtype
    mult = mybir.AluOpType.mult
    add = mybir.AluOpType.add

    nc = bacc.Bacc("TRN2", target_bir_lowering=False, debug=False, num_devices=NCORES)
    x = nc.dram_tensor("input", [NPC, C, H, W], fp32, kind="ExternalInput").ap()
    wgt = nc.dram_tensor("weight", [NPC, CC, KK, H, W], fp32, kind="ExternalInput").ap()
    y = nc.dram_tensor("output", [NPC, C, H, W], fp32, kind="ExternalOutput").ap()

    with tile.TileContext(nc) as tc:
        with (
            tc.tile_pool(name="main", bufs=1) as pool,
            tc.tile_pool(name="prod", bufs=2) as ppool,
            tc.tile_pool(name="rowp", bufs=3) as rpool,
            tc.tile_pool(name="tree", bufs=1) as tpool,
        ):
            in_pad = pool.tile([128, NPC, G, R, COLP], cdt)
            stage = pool.tile([128, NPC, G, R, W], fp32)
            w_t = pool.tile([128, NPC, KK, HB, W], cdt)
            acc = pool.tile([128, NPC, G, HB, W], fp32)
            if mode == "fp16row":
                in_pad1 = pool.tile([128, NPC, G, R, COLP], cdt)

            # Zero only the halo regions: left/right column borders of
            # in_pad/in_pad1, and the top/bottom staging row-bands that flow
            # into the padded rows. On the DVE: it is idle at kernel start and
            # clears these fast, unblocking the staging DMAs (WAW).
            # full-partition stage row bands first: they gate the staging DMAs
            # (the DMAs overwrite whichever rows are valid for their block)
            nc.vector.memset(stage[:, :, :, 0:PAD, :], 0.0)
            nc.vector.memset(stage[:, :, :, R - PAD : R, :], 0.0)
            nc.vector.memset(in_pad[:, :, :, :, 0:PAD], 0.0)
            nc.vector.memset(in_pad[:, :, :, :, PAD + W : COLP], 0.0)
            if mode == "fp16row":
                # in_pad1 holds input shifted one column left: interior
                # cols [PAD-1, PAD-1+W), borders outside that
                nc.vector.memset(in_pad1[:, :, :, :, 0 : PAD - 1], 0.0)
                nc.vector.memset(in_pad1[:, :, :, :, PAD - 1 + W : COLP], 0.0)

            # Input staging loads on HWDGE (fp32), then two independent
            # convert-copies of stage: ACT builds in_pad (interior at col PAD),
            # DVE builds in_pad1 (interior at col PAD-1, i.e. shifted one
            # column left, which keeps odd-kw taps 4B-aligned for the DVE 2x
            # perf mode). Weights stream via SWDGE dtype-cast DMAs in two
            # kh-chunks so compute can start after chunk A lands.
            for n in range(NPC):
                for blk in range(BLK):
                    # rows of the padded window [blk*8-3, blk*8+11) that exist
                    h0 = max(0, blk * HB - PAD)
                    h1 = min(H, blk * HB + HB + PAD)
                    r0 = h0 - (blk * HB - PAD)
                    dst = stage[blk::BLK, n].rearrange("p g r c -> p g (r c)")[
                        :, :, r0 * W : (r0 + (h1 - h0)) * W
                    ]
                    src = x[n].rearrange("(g cc) h w -> cc g (h w)", g=G)[
                        :, :, h0 * W : h1 * W
                    ]
                    nc.sync.dma_start(out=dst, in_=src)
            for n in range(NPC):
                nc.scalar.copy(out=in_pad[:, n, :, :, PAD : PAD + W], in_=stage[:, n])
                if mode == "fp16row":
                    nc.vector.tensor_copy(
                        out=in_pad1[:, n, :, :, PAD - 1 : PAD - 1 + W],
                        in_=stage[:, n],
                    )
            KH_CHUNKS = [(0, 1), (1, 4), (4, K)] if mode == "fp16row" else [(0, K)]
            for kh0, kh1 in KH_CHUNKS:
                for n in range(NPC):
                    wsrc = wgt[n].rearrange(
                        "cc kk (blk hb) w -> cc blk kk (hb w)", blk=BLK
                    )
                    for blk in range(BLK):
                        nc.gpsimd.dma_start(
                            out=w_t[blk::BLK, n, kh0 * K : kh1 * K].rearrange(
                                "p kk hb w -> p kk (hb w)"
                            ),
                            in_=wsrc[:, blk, kh0 * K : kh1 * K],
                        )

            if mode == "fp32":
                for n in range(NPC):
                    for kh in range(K):
                        for kw in range(K):
                            kk = kh * K + kw
                            in0 = in_pad[:, n, :, kh : kh + HB, kw : kw + W]
                            in1 = (
                                w_t[:, n, kk].unsqueeze(1).broadcast_to([128, G, HB, W])
                            )
                            if kk == 0:
                                nc.vector.tensor_tensor(
                                    out=acc[:, n], in0=in0, in1=in1, op=mult
                                )
                            else:
                                prod = ppool.tile([128, G, HB, W], cdt)
                                nc.vector.tensor_tensor(
                                    out=prod[:], in0=in0, in1=in1, op=mult
                                )
                                nc.vector.tensor_tensor(
                                    out=acc[:, n], in0=acc[:, n], in1=prod[:], op=add
                                )
            else:
                # both images processed by each instruction (n on a free axis)
                shp = [128, NPC, G, HB, W]
                prev_row = None
                treeacc = tpool.tile(shp, cdt, tag="treeacc")
                for kh in range(K):
                    rowacc = rpool.tile(shp, cdt)
                    for kw in range(K):
                        kk = kh * K + kw
                        if kw % 2 == 0:
                            in0 = in_pad[:, :, :, kh : kh + HB, kw : kw + W]
                        else:
                            in0 = in_pad1[:, :, :, kh : kh + HB, kw - 1 : kw - 1 + W]
                        in1 = w_t[:, :, kk].unsqueeze(2).broadcast_to(shp)
                        if kw == 0:
                            nc.vector.tensor_tensor(
                                out=rowacc[:], in0=in0, in1=in1, op=mult
                            )
                        else:
                            prod = ppool.tile(shp, cdt)
                            nc.vector.tensor_tensor(
                                out=prod[:], in0=in0, in1=in1, op=mult
                            )
                            nc.vector.tensor_tensor(
                                out=rowacc[:], in0=rowacc[:], in1=prod[:], op=add
                            )
                    # pairwise fp16 combine of row sums (keeps the fp16 2x
                    # rate; only the very last add writes fp32)
                    if kh == K - 1:
                        # split by g-half so the first half's output stores can
                        # overlap the second half's add
                        half = G // 2
                        nc.vector.tensor_tensor(
                            out=acc[:, :, 0:half],
                            in0=treeacc[:, :, 0:half],
                            in1=rowacc[:, :, 0:half],
                            op=add,
                        )
                        nc.vector.tensor_tensor(
                            out=acc[:, :, half:G],
                            in0=treeacc[:, :, half:G],
                            in1=rowacc[:, :, half:G],
                            op=add,
                        )
                    elif kh % 2 == 1:
                        if kh == 1:
                            nc.vector.tensor_tensor(
                                out=treeacc[:], in0=prev_row[:], in1=rowacc[:], op=add
                            )
                        else:
                            pair = tpool.tile(shp, cdt, tag="pair")
                            nc.vector.tensor_tensor(
                                out=pair[:], in0=prev_row[:], in1=rowacc[:], op=add
                            )
                            nc.vector.tensor_tensor(
                                out=treeacc[:], in0=treeacc[:], in1=pair[:], op=add
                            )
                        prev_row = None
                    else:
                        prev_row = rowacc
            for g in range(G):
                for n in range(NPC):
                    dsty = y[n].rearrange(
                        "(g cc) (blk hb) w -> g cc blk (hb w)", g=G, blk=BLK
                    )
                    nc.sync.dma_start(
                        out=dsty[g],
                        in_=acc[:, n, g].rearrange("p hb w -> p (hb w)"),
                    )

    nc.compile()
    return nc


def _get_nc(mode=None):
    mode = mode or MODE
    if mode not in _cache:
        _cache[mode] = _build(mode)
    return _cache[mode]


def kernel(input_, weight, _trace=False, _mode=None):
    from concourse.bass_utils import run_bass_kernel_spmd

    nc = _get_nc(_mode)
    input_ = np.ascontiguousarray(input_, dtype=np.float32)
    weight = np.ascontiguousarray(weight, dtype=np.float32)
    in_maps = [
        {
            "input": input_[i * NPC : (i + 1) * NPC],
            "weight": weight[i * NPC : (i + 1) * NPC],
        }
        for i in range(NCORES)
    ]
    res = run_bass_kernel_spmd(nc, in_maps, list(range(NCORES)), trace=_trace)
    _cache["last_result"] = res
    out = np.concatenate([res.results[i]["output"] for i in range(NCORES)], axis=0)
    return out


# revision 24
# speedup vs baseline: 1.0562x; 1.0562x over previous
"""Trainium2 Bass kernel for nn_Aggregation (SAN-style local aggregation).

out[n, g*32+cc, h, w] = sum_{kh,kw} input[n, g*32+cc, h-3+kh, w-3+kw] * weight[n, cc, kh*7+kw, h, w]

Sharding: data-parallel over batch N=16 across 8 NeuronCores (2 images/core).

Per-core layout:
  partition p = cc*4 + blk   (cc in [0,32): weight channel, blk in [0,4): block of 8 output rows)
  in_pad[p][n, g, r, col] = zero-padded input rows [blk*8-3, blk*8+11), cols [-3, 35)
  w_t[p][n, kk, hb, w]    = weight[n, cc, kk, blk*8+hb, w]
  For each tap kk=(kh,kw): acc[p][n,g,hb,w] += in_pad[p][n,g,hb+kh,w+kw] * w_t[p][n,kk,hb,w]
  (weight broadcast over g via stride-0 access pattern)

Mode "fp16row" (default): products and within-row (7-tap) accumulation in fp16
on the DVE at 2x rate; row sums flushed into an fp32 accumulator. A second
input copy shifted by one column keeps odd-kw taps 4B-aligned so the DVE's
2x perf mode stays engaged. Max abs error vs fp32 reference ~7e-4 of absmax.
Mode "fp32": everything fp32 (exact, ~2x slower).
"""

import numpy as np

N, C, H, W = 16, 256, 32, 32
K, PAD = 7, 3
CC, G = 32, 8
KK = K * K
NCORES = 8
NPC = N // NCORES
BLK, HB = 4, 8
R, COLP = HB + 2 * PAD, W + 2 * PAD  # 14, 38

MODE = "fp16row"
WCHUNK = False

_cache = {}


def _build(mode):
    import concourse.bacc as bacc
    import concourse.mybir as mybir
    import concourse.tile as tile

    fp32 = mybir.dt.float32
    fp16 = mybir.dt.float16
    cdt = fp32 if mode == "fp32" else fp16  # compute dtype
    mult = mybir.AluOpType.mult
    add = mybir.AluOpType.add

    nc = bacc.Bacc("TRN2", target_bir_lowering=False, debug=False, num_devices=NCORES)
    x = nc.dram_tensor("input", [NPC, C, H, W], fp32, kind="ExternalInput").ap()
    wgt = nc.dram_tensor("weight", [NPC, CC, KK, H, W], fp32, kind="ExternalInput").ap()
    y = nc.dram_tensor("output", [NPC, C, H, W], fp32, kind="ExternalOutput").ap()

    with tile.TileContext(nc) as tc:
        with (
            tc.tile_pool(name="main", bufs=1) as pool,
            tc.tile_pool(name="prod", bufs=2) as ppool,
            tc.tile_pool(name="rowp", bufs=3) as rpool,
            tc.tile_pool(name="tree", bufs=1) as tpool,
        ):
            in_pad = pool.tile([128, NPC, G, R, COLP], cdt)
            stage = pool.tile([128, NPC, G, R, W], fp32)
            w_t = pool.tile([128, NPC, KK, HB, W], cdt)
            acc = pool.tile([128, NPC, G, HB, W], fp32)
            if mode == "fp16row":
                in_pad1 = pool.tile([128, NPC, G, R, COLP], cdt)

            # Zero only the halo regions: left/right column borders of
            # in_pad/in_pad1, and the top/bottom staging row-bands that flow
            # into the padded rows. On the DVE: it is idle at kernel start and
            # clears these fast, unblocking the staging DMAs (WAW).
            # full-partition stage row bands first: they gate the staging DMAs
            # (the DMAs overwrite whichever rows are valid for their block)
            nc.vector.memset(stage[:, :, :, 0:PAD, :], 0.0)
            nc.vector.memset(stage[:, :, :, R - PAD : R, :], 0.0)
            nc.vector.memset(in_pad[:, :, :, :, 0:PAD], 0.0)
            nc.vector.memset(in_pad[:, :, :, :, PAD + W : COLP], 0.0)
            if mode == "fp16row":
                # in_pad1 holds input shifted one column left: interior
                # cols [PAD-1, PAD-1+W), borders outside that
                nc.vector.memset(in_pad1[:, :, :, :, 0 : PAD - 1], 0.0)
                nc.vector.memset(in_pad1[:, :, :, :, PAD - 1 + W : COLP], 0.0)

            # Input staging loads on HWDGE (fp32), then two independent
            # convert-copies of stage: ACT builds in_pad (interior at col PAD),
            # DVE builds in_pad1 (interior at col PAD-1, i.e. shifted one
            # column left, which keeps odd-kw taps 4B-aligned for the DVE 2x
            # perf mode). Weights stream via SWDGE dtype-cast DMAs in two
            # kh-chunks so compute can start after chunk A lands.
            for n in range(NPC):
                for blk in range(BLK):
                    # rows of the padded window [blk*8-3, blk*8+11) that exist
                    h0 = max(0, blk * HB - PAD)
                    h1 = min(H, blk * HB + HB + PAD)
                    r0 = h0 - (blk * HB - PAD)
                    dst = stage[blk::BLK, n].rearrange("p g r c -> p g (r c)")[
                        :, :, r0 * W : (r0 + (h1 - h0)) * W
                    ]
                    src = x[n].rearrange("(g cc) h w -> cc g (h w)", g=G)[
                        :, :, h0 * W : h1 * W
                    ]
                    nc.sync.dma_start(out=dst, in_=src)
            for n in range(NPC):
                nc.scalar.copy(out=in_pad[:, n, :, :, PAD : PAD + W], in_=stage[:, n])
                if mode == "fp16row":
                    nc.vector.tensor_copy(
                        out=in_pad1[:, n, :, :, PAD - 1 : PAD - 1 + W],
                        in_=stage[:, n],
                    )
            KH_CHUNKS = [(0, 2), (2, 5), (5, K)] if mode == "fp16row" else [(0, K)]
            for kh0, kh1 in KH_CHUNKS:
                for n in range(NPC):
                    wsrc = wgt[n].rearrange(
                        "cc kk (blk hb) w -> cc blk kk (hb w)", blk=BLK
                    )
                    for blk in range(BLK):
                        nc.gpsimd.dma_start(
                            out=w_t[blk::BLK, n, kh0 * K : kh1 * K].rearrange(
                                "p kk hb w -> p kk (hb w)"
                            ),
                            in_=wsrc[:, blk, kh0 * K : kh1 * K],
                        )

            if mode == "fp32":
                for n in range(NPC):
                    for kh in range(K):
                        for kw in range(K):
                            kk = kh * K + kw
                            in0 = in_pad[:, n, :, kh : kh + HB, kw : kw + W]
                            in1 = (
                                w_t[:, n, kk].unsqueeze(1).broadcast_to([128, G, HB, W])
                            )
                            if kk == 0:
                                nc.vector.tensor_tensor(
                                    out=acc[:, n], in0=in0, in1=in1, op=mult
                                )
                            else:
                                prod = ppool.tile([128, G, HB, W], cdt)
                                nc.vector.tensor_tensor(
                                    out=prod[:], in0=in0, in1=in1, op=mult
                                )
                                nc.vector.tensor_tensor(
                                    out=acc[:, n], in0=acc[:, n], in1=prod[:], op=add
                                )
            else:
                # both images processed by each instruction (n on a free axis)
                shp = [128, NPC, G, HB, W]
                prev_row = None
                treeacc = tpool.tile(shp, cdt, tag="treeacc")
                for kh in range(K):
                    rowacc = rpool.tile(shp, cdt)
                    for kw in range(K):
                        kk = kh * K + kw
                        if kw % 2 == 0:
                            in0 = in_pad[:, :, :, kh : kh + HB, kw : kw + W]
                        else:
                            in0 = in_pad1[:, :, :, kh : kh + HB, kw - 1 : kw - 1 + W]
                        in1 = w_t[:, :, kk].unsqueeze(2).broadcast_to(shp)
                        if kw == 0:
                            nc.vector.tensor_tensor(
                                out=rowacc[:], in0=in0, in1=in1, op=mult
                            )
                        else:
                            prod = ppool.tile(shp, cdt)
                            nc.vector.tensor_tensor(
                                out=prod[:], in0=in0, in1=in1, op=mult
                            )
                            nc.vector.tensor_tensor(
                                out=rowacc[:], in0=rowacc[:], in1=prod[:], op=add
                            )
                    # pairwise fp16 combine of row sums (keeps the fp16 2x
                    # rate; only the very last add writes fp32)
                    if kh == K - 1:
                        # split by g-half so the first half's output stores can
                        # overlap the second half's add
                        half = G // 2
                        nc.vector.tensor_tensor(
                            out=acc[:, :, 0:half],
                            in0=treeacc[:, :, 0:half],
                            in1=rowacc[:, :, 0:half],
                            op=add,
                        )
                        nc.vector.tensor_tensor(
                            out=acc[:, :, half:G],
                            in0=treeacc[:, :, half:G],
                            in1=rowacc[:, :, half:G],
                            op=add,
                        )
                    elif kh % 2 == 1:
                        if kh == 1:
                            nc.vector.tensor_tensor(
                                out=treeacc[:], in0=prev_row[:], in1=rowacc[:], op=add
                            )
                        else:
                            pair = tpool.tile(shp, cdt, tag="pair")
                            nc.vector.tensor_tensor(
                                out=pair[:], in0=prev_row[:], in1=rowacc[:], op=add
                            )
                            nc.vector.tensor_tensor(
                                out=treeacc[:], in0=treeacc[:], in1=pair[:], op=add
                            )
                        prev_row = None
                    else:
                        prev_row = rowacc
            for g in range(G):
                for n in range(NPC):
                    dsty = y[n].rearrange(
                        "(g cc) (blk hb) w -> g cc blk (hb w)", g=G, blk=BLK
                    )
                    nc.sync.dma_start(
                        out=dsty[g],
                        in_=acc[:, n, g].rearrange("p hb w -> p (hb w)"),
                    )

    nc.compile()
    return nc


def _get_nc(mode=None):
    mode = mode or MODE
    if mode not in _cache:
        _cache[mode] = _build(mode)
    return _cache[mode]


def kernel(input_, weight, _trace=False, _mode=None):
    from concourse.bass_utils import run_bass_kernel_spmd

    nc = _get_nc(_mode)
    input_ = np.ascontiguousarray(input_, dtype=np.float32)
    weight = np.ascontiguousarray(weight, dtype=np.float32)
    in_maps = [
        {
            "input": input_[i * NPC : (i + 1) * NPC],
            "weight": weight[i * NPC : (i + 1) * NPC],
        }
        for i in range(NCORES)
    ]
    res = run_bass_kernel_spmd(nc, in_maps, list(range(NCORES)), trace=_trace)
    _cache["last_result"] = res
    out = np.concatenate([res.results[i]["output"] for i in range(NCORES)], axis=0)
    return out


# revision 30
# speedup vs baseline: 1.4693x; 1.3912x over previous
"""Trainium2 Bass kernel for nn_Aggregation (SAN-style local aggregation).

out[n, g*32+cc, h, w] = sum_{kh,kw} input[n, g*32+cc, h-3+kh, w-3+kw] * weight[n, cc, kh*7+kw, h, w]

Sharding: data-parallel over batch N=16 across 8 NeuronCores (2 images/core).

Per-core layout:
  partition p = cc*4 + blk   (cc in [0,32): weight channel, blk in [0,4): block of 8 output rows)
  in_pad[p][n, g, r, col] = zero-padded input rows [blk*8-3, blk*8+11), cols [-3, 35)
  w_t[p][n, kk, hb, w]    = weight[n, cc, kk, blk*8+hb, w]
  For each tap kk=(kh,kw): acc[p][n,g,hb,w] += in_pad[p][n,g,hb+kh,w+kw] * w_t[p][n,kk,hb,w]
  (weight broadcast over g via stride-0 access pattern)

Mode "fp16row" (default): products and within-row (7-tap) accumulation in fp16
on the DVE at 2x rate; row sums flushed into an fp32 accumulator. A second
input copy shifted by one column keeps odd-kw taps 4B-aligned so the DVE's
2x perf mode stays engaged. Max abs error vs fp32 reference ~7e-4 of absmax.
Mode "fp32": everything fp32 (exact, ~2x slower).
"""

import numpy as np

N, C, H, W = 16, 256, 32, 32
K, PAD = 7, 3
CC, G = 32, 8
KK = K * K
NCORES = 8
NPC = N // NCORES
BLK, HB = 4, 8
R, COLP = HB + 2 * PAD, W + 2 * PAD  # 14, 38

MODE = "fp16pe"

_cache = {}


def _build(mode):
    import concourse.bacc as bacc
    import concourse.mybir as mybir
    import concourse.tile as tile

    fp32 = mybir.dt.float32
    fp16 = mybir.dt.float16
    cdt = fp32 if mode == "fp32" else fp16  # compute dtype
    mult = mybir.AluOpType.mult
    add = mybir.AluOpType.add

    nc = bacc.Bacc("TRN2", target_bir_lowering=False, debug=False, num_devices=NCORES)
    x = nc.dram_tensor("input", [NPC, C, H, W], fp32, kind="ExternalInput").ap()
    wgt = nc.dram_tensor("weight", [NPC, CC, KK, H, W], fp32, kind="ExternalInput").ap()
    y = nc.dram_tensor("output", [NPC, C, H, W], fp32, kind="ExternalOutput").ap()
    if mode == "fp16pe":
        idn = nc.dram_tensor("identity", [128, 128], fp16, kind="ExternalInput").ap()

    with tile.TileContext(nc) as tc:
        with (
            tc.tile_pool(name="main", bufs=1) as pool,
            tc.tile_pool(name="prod", bufs=2) as ppool,
            tc.tile_pool(name="rowp", bufs=3) as rpool,
            tc.tile_pool(name="tree", bufs=1) as tpool,
            tc.tile_pool(name="psum", bufs=1, space="PSUM") as pspool,
        ):
            in_pad = pool.tile([128, NPC, G, R, COLP], cdt)
            stage = pool.tile([128, NPC, G, R, W], fp32)
            w_t = pool.tile([128, NPC, KK, HB, W], cdt)
            acc = pool.tile([128, NPC, G, HB, W], fp32)
            if mode != "fp32":
                in_pad1 = pool.tile([128, NPC, G, R, COLP], cdt)
            if mode == "fp16pe":
                ident = pool.tile([128, 128], cdt)
                nc.sync.dma_start(out=ident[:], in_=idn[:])
                acc_ps = pspool.tile([128, NPC * G * HB * W], fp32)

            # Zero only the halo regions: left/right column borders of
            # in_pad/in_pad1, and the top/bottom staging row-bands that flow
            # into the padded rows. On the DVE: it is idle at kernel start and
            # clears these fast, unblocking the staging DMAs (WAW).
            # full-partition stage row bands first: they gate the staging DMAs
            # (the DMAs overwrite whichever rows are valid for their block)
            nc.vector.memset(stage[:, :, :, 0:PAD, :], 0.0)
            nc.vector.memset(stage[:, :, :, R - PAD : R, :], 0.0)
            nc.vector.memset(in_pad[:, :, :, :, 0:PAD], 0.0)
            nc.vector.memset(in_pad[:, :, :, :, PAD + W : COLP], 0.0)
            if mode != "fp32":
                # in_pad1 holds input shifted one column left: interior
                # cols [PAD-1, PAD-1+W), borders outside that
                nc.vector.memset(in_pad1[:, :, :, :, 0 : PAD - 1], 0.0)
                nc.vector.memset(in_pad1[:, :, :, :, PAD - 1 + W : COLP], 0.0)

            # Input staging loads on HWDGE (fp32), then two independent
            # convert-copies of stage: ACT builds in_pad (interior at col PAD),
            # DVE builds in_pad1 (interior at col PAD-1, i.e. shifted one
            # column left, which keeps odd-kw taps 4B-aligned for the DVE 2x
            # perf mode). Weights stream via SWDGE dtype-cast DMAs in two
            # kh-chunks so compute can start after chunk A lands.
            for n in range(NPC):
                for blk in range(BLK):
                    # rows of the padded window [blk*8-3, blk*8+11) that exist
                    h0 = max(0, blk * HB - PAD)
                    h1 = min(H, blk * HB + HB + PAD)
                    r0 = h0 - (blk * HB - PAD)
                    dst = stage[blk::BLK, n].rearrange("p g r c -> p g (r c)")[
                        :, :, r0 * W : (r0 + (h1 - h0)) * W
                    ]
                    src = x[n].rearrange("(g cc) h w -> cc g (h w)", g=G)[
                        :, :, h0 * W : h1 * W
                    ]
                    nc.sync.dma_start(out=dst, in_=src)
            for n in range(NPC):
                nc.scalar.copy(out=in_pad[:, n, :, :, PAD : PAD + W], in_=stage[:, n])
                if mode != "fp32":
                    nc.vector.tensor_copy(
                        out=in_pad1[:, n, :, :, PAD - 1 : PAD - 1 + W],
                        in_=stage[:, n],
                    )
            KH_CHUNKS = [(0, 2), (2, 5), (5, K)] if mode != "fp32" else [(0, K)]
            for kh0, kh1 in KH_CHUNKS:
                for n in range(NPC):
                    wsrc = wgt[n].rearrange(
                        "cc kk (blk hb) w -> cc blk kk (hb w)", blk=BLK
                    )
                    for blk in range(BLK):
                        nc.gpsimd.dma_start(
                            out=w_t[blk::BLK, n, kh0 * K : kh1 * K].rearrange(
                                "p kk hb w -> p kk (hb w)"
                            ),
                            in_=wsrc[:, blk, kh0 * K : kh1 * K],
                        )

            if mode == "fp32":
                for n in range(NPC):
                    for kh in range(K):
                        for kw in range(K):
                            kk = kh * K + kw
                            in0 = in_pad[:, n, :, kh : kh + HB, kw : kw + W]
                            in1 = (
                                w_t[:, n, kk].unsqueeze(1).broadcast_to([128, G, HB, W])
                            )
                            if kk == 0:
                                nc.vector.tensor_tensor(
                                    out=acc[:, n], in0=in0, in1=in1, op=mult
                                )
                            else:
                                prod = ppool.tile([128, G, HB, W], cdt)
                                nc.vector.tensor_tensor(
                                    out=prod[:], in0=in0, in1=in1, op=mult
                                )
                                nc.vector.tensor_tensor(
                                    out=acc[:, n], in0=acc[:, n], in1=prod[:], op=add
                                )
            elif mode == "fp16pe":
                # DVE computes fp16 products at its 2x rate; the otherwise
                # idle Tensor engine accumulates them into an fp32 PSUM
                # accumulator via identity matmuls (start clears, subsequent
                # matmuls accumulate via PSUM has_written bits). Accumulation
                # is therefore exact fp32: only products are rounded to fp16.
                shp = [128, NPC, G, HB, W]
                FLAT = NPC * G * HB * W  # 4096 fp32 = exactly 8 PSUM banks
                NBANK = FLAT // 512
                for kk in range(KK):
                    kh, kw = divmod(kk, K)
                    if kw % 2 == 0:
                        in0 = in_pad[:, :, :, kh : kh + HB, kw : kw + W]
                    else:
                        in0 = in_pad1[:, :, :, kh : kh + HB, kw - 1 : kw - 1 + W]
                    in1 = w_t[:, :, kk].unsqueeze(2).broadcast_to(shp)
                    prod = ppool.tile(shp, cdt)
                    nc.vector.tensor_tensor(out=prod[:], in0=in0, in1=in1, op=mult)
                    pf = prod[:].rearrange("p n g h w -> p (n g h w)")
                    for b in range(NBANK):
                        nc.tensor.matmul(
                            out=acc_ps[:, 512 * b : 512 * (b + 1)],
                            lhsT=ident[:],
                            rhs=pf[:, 512 * b : 512 * (b + 1)],
                            start=(kk == 0),
                            stop=(kk == KK - 1),
                        )
                # evict PSUM -> SBUF (fp32) in two halves on ACT so the first
                # half's stores overlap the second half's copy
                av = acc[:].rearrange("p n g h w -> p (n g h w)")
                nc.scalar.copy(out=av[:, : FLAT // 2], in_=acc_ps[:, : FLAT // 2])
                nc.scalar.copy(out=av[:, FLAT // 2 :], in_=acc_ps[:, FLAT // 2 :])
            else:
                # both images processed by each instruction (n on a free axis)
                shp = [128, NPC, G, HB, W]
                prev_row = None
                treeacc = tpool.tile(shp, cdt, tag="treeacc")
                for kh in range(K):
                    rowacc = rpool.tile(shp, cdt)
                    for kw in range(K):
                        kk = kh * K + kw
                        if kw % 2 == 0:
                            in0 = in_pad[:, :, :, kh : kh + HB, kw : kw + W]
                        else:
                            in0 = in_pad1[:, :, :, kh : kh + HB, kw - 1 : kw - 1 + W]
                        in1 = w_t[:, :, kk].unsqueeze(2).broadcast_to(shp)
                        if kw == 0:
                            nc.vector.tensor_tensor(
                                out=rowacc[:], in0=in0, in1=in1, op=mult
                            )
                        else:
                            prod = ppool.tile(shp, cdt)
                            nc.vector.tensor_tensor(
                                out=prod[:], in0=in0, in1=in1, op=mult
                            )
                            nc.vector.tensor_tensor(
                                out=rowacc[:], in0=rowacc[:], in1=prod[:], op=add
                            )
                    # pairwise fp16 combine of row sums (keeps the fp16 2x
                    # rate; only the very last add writes fp32)
                    if kh == K - 1:
                        # split by g-half so the first half's output stores can
                        # overlap the second half's add
                        half = G // 2
                        nc.vector.tensor_tensor(
                            out=acc[:, :, 0:half],
                            in0=treeacc[:, :, 0:half],
                            in1=rowacc[:, :, 0:half],
                            op=add,
                        )
                        nc.vector.tensor_tensor(
                            out=acc[:, :, half:G],
                            in0=treeacc[:, :, half:G],
                            in1=rowacc[:, :, half:G],
                            op=add,
                        )
                    elif kh % 2 == 1:
                        if kh == 1:
                            nc.vector.tensor_tensor(
                                out=treeacc[:], in0=prev_row[:], in1=rowacc[:], op=add
                            )
                        else:
                            pair = tpool.tile(shp, cdt, tag="pair")
                            nc.vector.tensor_tensor(
                                out=pair[:], in0=prev_row[:], in1=rowacc[:], op=add
                            )
                            nc.vector.tensor_tensor(
                                out=treeacc[:], in0=treeacc[:], in1=pair[:], op=add
                            )
                        prev_row = None
                    else:
                        prev_row = rowacc
            for g in range(G):
                for n in range(NPC):
                    dsty = y[n].rearrange(
                        "(g cc) (blk hb) w -> g cc blk (hb w)", g=G, blk=BLK
                    )
                    nc.sync.dma_start(
                        out=dsty[g],
                        in_=acc[:, n, g].rearrange("p hb w -> p (hb w)"),
                    )

    nc.compile()
    return nc


def _get_nc(mode=None):
    mode = mode or MODE
    if mode not in _cache:
        _cache[mode] = _build(mode)
    return _cache[mode]


def kernel(input_, weight, _trace=False, _mode=None):
    from concourse.bass_utils import run_bass_kernel_spmd

    mode = _mode or MODE
    nc = _get_nc(mode)
    input_ = np.ascontiguousarray(input_, dtype=np.float32)
    weight = np.ascontiguousarray(weight, dtype=np.float32)
    eye = np.eye(128, dtype=np.float16)
    in_maps = [
        {
            "input": input_[i * NPC : (i + 1) * NPC],
            "weight": weight[i * NPC : (i + 1) * NPC],
            **({"identity": eye} if mode == "fp16pe" else {}),
        }
        for i in range(NCORES)
    ]
    res = run_bass_kernel_spmd(nc, in_maps, list(range(NCORES)), trace=_trace)
    _cache["last_result"] = res
    out = np.concatenate([res.results[i]["output"] for i in range(NCORES)], axis=0)
    return out


# revision 32
# speedup vs baseline: 1.7337x; 1.1799x over previous
"""Trainium2 Bass kernel for nn_Aggregation (SAN-style local aggregation).

out[n, g*32+cc, h, w] = sum_{kh,kw} input[n, g*32+cc, h-3+kh, w-3+kw] * weight[n, cc, kh*7+kw, h, w]

Sharding: data-parallel over batch N=16 across 8 NeuronCores (2 images/core).

Per-core layout:
  partition p = cc*4 + blk   (cc in [0,32): weight channel, blk in [0,4): block of 8 output rows)
  in_pad[p][n, g, r, col] = zero-padded input rows [blk*8-3, blk*8+11), cols [-3, 35)
  w_t[p][n, kk, hb, w]    = weight[n, cc, kk, blk*8+hb, w]
  For each tap kk=(kh,kw): acc[p][n,g,hb,w] += in_pad[p][n,g,hb+kh,w+kw] * w_t[p][n,kk,hb,w]
  (weight broadcast over g via stride-0 access pattern)

Mode "fp16row" (default): products and within-row (7-tap) accumulation in fp16
on the DVE at 2x rate; row sums flushed into an fp32 accumulator. A second
input copy shifted by one column keeps odd-kw taps 4B-aligned so the DVE's
2x perf mode stays engaged. Max abs error vs fp32 reference ~7e-4 of absmax.
Mode "fp32": everything fp32 (exact, ~2x slower).
"""

import numpy as np

N, C, H, W = 16, 256, 32, 32
K, PAD = 7, 3
CC, G = 32, 8
KK = K * K
NCORES = 8
NPC = N // NCORES
BLK, HB = 4, 8
R, COLP = HB + 2 * PAD, W + 2 * PAD  # 14, 38

MODE = "fp16pe"

_cache = {}


def _build(mode):
    import concourse.bacc as bacc
    import concourse.mybir as mybir
    import concourse.tile as tile

    fp32 = mybir.dt.float32
    fp16 = mybir.dt.float16
    cdt = fp32 if mode == "fp32" else fp16  # compute dtype
    mult = mybir.AluOpType.mult
    add = mybir.AluOpType.add

    nc = bacc.Bacc("TRN2", target_bir_lowering=False, debug=False, num_devices=NCORES)
    x = nc.dram_tensor("input", [NPC, C, H, W], fp32, kind="ExternalInput").ap()
    wgt = nc.dram_tensor("weight", [NPC, CC, KK, H, W], fp32, kind="ExternalInput").ap()
    y = nc.dram_tensor("output", [NPC, C, H, W], fp32, kind="ExternalOutput").ap()
    if mode == "fp16pe":
        idn = nc.dram_tensor("identity", [128, 128], fp16, kind="ExternalInput").ap()

    with tile.TileContext(nc) as tc:
        with (
            tc.tile_pool(name="main", bufs=1) as pool,
            tc.tile_pool(name="prod", bufs=4) as ppool,
            tc.tile_pool(name="rowp", bufs=3) as rpool,
            tc.tile_pool(name="tree", bufs=1) as tpool,
            tc.tile_pool(name="psum", bufs=1, space="PSUM") as pspool,
        ):
            in_pad = pool.tile([128, NPC, G, R, COLP], cdt)
            stage = pool.tile([128, NPC, G, R, W], fp32)
            w_t = pool.tile([128, NPC, KK, HB, W], cdt)
            acc = pool.tile([128, NPC, G, HB, W], fp32)
            if mode != "fp32":
                in_pad1 = pool.tile([128, NPC, G, R, COLP], cdt)
            if mode == "fp16pe":
                ident = pool.tile([128, 128], cdt)
                nc.sync.dma_start(out=ident[:], in_=idn[:])
                acc_ps = pspool.tile([128, NPC * G * HB * W], fp32)

            # Zero only the halo regions: left/right column borders of
            # in_pad/in_pad1, and the top/bottom staging row-bands that flow
            # into the padded rows. On the DVE: it is idle at kernel start and
            # clears these fast, unblocking the staging DMAs (WAW).
            # full-partition stage row bands first: they gate the staging DMAs
            # (the DMAs overwrite whichever rows are valid for their block)
            nc.vector.memset(stage[:, :, :, 0:PAD, :], 0.0)
            nc.vector.memset(stage[:, :, :, R - PAD : R, :], 0.0)
            nc.vector.memset(in_pad[:, :, :, :, 0:PAD], 0.0)
            nc.vector.memset(in_pad[:, :, :, :, PAD + W : COLP], 0.0)
            if mode != "fp32":
                # in_pad1 holds input shifted one column left: interior
                # cols [PAD-1, PAD-1+W), borders outside that
                nc.vector.memset(in_pad1[:, :, :, :, 0 : PAD - 1], 0.0)
                nc.vector.memset(in_pad1[:, :, :, :, PAD - 1 + W : COLP], 0.0)

            # Input staging loads on HWDGE (fp32), then two independent
            # convert-copies of stage: ACT builds in_pad (interior at col PAD),
            # DVE builds in_pad1 (interior at col PAD-1, i.e. shifted one
            # column left, which keeps odd-kw taps 4B-aligned for the DVE 2x
            # perf mode). Weights stream via SWDGE dtype-cast DMAs in two
            # kh-chunks so compute can start after chunk A lands.
            for n in range(NPC):
                for blk in range(BLK):
                    # rows of the padded window [blk*8-3, blk*8+11) that exist
                    h0 = max(0, blk * HB - PAD)
                    h1 = min(H, blk * HB + HB + PAD)
                    r0 = h0 - (blk * HB - PAD)
                    dst = stage[blk::BLK, n].rearrange("p g r c -> p g (r c)")[
                        :, :, r0 * W : (r0 + (h1 - h0)) * W
                    ]
                    src = x[n].rearrange("(g cc) h w -> cc g (h w)", g=G)[
                        :, :, h0 * W : h1 * W
                    ]
                    nc.sync.dma_start(out=dst, in_=src)
            for n in range(NPC):
                nc.scalar.copy(out=in_pad[:, n, :, :, PAD : PAD + W], in_=stage[:, n])
                if mode != "fp32":
                    nc.vector.tensor_copy(
                        out=in_pad1[:, n, :, :, PAD - 1 : PAD - 1 + W],
                        in_=stage[:, n],
                    )
            KH_CHUNKS = [(0, 2), (2, 5), (5, K)] if mode != "fp32" else [(0, K)]
            for kh0, kh1 in KH_CHUNKS:
                for n in range(NPC):
                    wsrc = wgt[n].rearrange(
                        "cc kk (blk hb) w -> cc blk kk (hb w)", blk=BLK
                    )
                    for blk in range(BLK):
                        nc.gpsimd.dma_start(
                            out=w_t[blk::BLK, n, kh0 * K : kh1 * K].rearrange(
                                "p kk hb w -> p kk (hb w)"
                            ),
                            in_=wsrc[:, blk, kh0 * K : kh1 * K],
                        )

            if mode == "fp32":
                for n in range(NPC):
                    for kh in range(K):
                        for kw in range(K):
                            kk = kh * K + kw
                            in0 = in_pad[:, n, :, kh : kh + HB, kw : kw + W]
                            in1 = (
                                w_t[:, n, kk].unsqueeze(1).broadcast_to([128, G, HB, W])
                            )
                            if kk == 0:
                                nc.vector.tensor_tensor(
                                    out=acc[:, n], in0=in0, in1=in1, op=mult
                                )
                            else:
                                prod = ppool.tile([128, G, HB, W], cdt)
                                nc.vector.tensor_tensor(
                                    out=prod[:], in0=in0, in1=in1, op=mult
                                )
                                nc.vector.tensor_tensor(
                                    out=acc[:, n], in0=acc[:, n], in1=prod[:], op=add
                                )
            elif mode == "fp16pe":
                # DVE computes fp16 products at its 2x rate; the otherwise
                # idle Tensor engine accumulates them into an fp32 PSUM
                # accumulator via identity matmuls (start clears, subsequent
                # matmuls accumulate via PSUM has_written bits). Accumulation
                # is therefore exact fp32: only products are rounded to fp16.
                shp = [128, NPC, G, HB, W]
                FLAT = NPC * G * HB * W  # 4096 fp32 = exactly 8 PSUM banks
                NBANK = FLAT // 512
                for kk in range(KK):
                    kh, kw = divmod(kk, K)
                    if kw % 2 == 0:
                        in0 = in_pad[:, :, :, kh : kh + HB, kw : kw + W]
                    else:
                        in0 = in_pad1[:, :, :, kh : kh + HB, kw - 1 : kw - 1 + W]
                    in1 = w_t[:, :, kk].unsqueeze(2).broadcast_to(shp)
                    prod = ppool.tile(shp, cdt)
                    nc.vector.tensor_tensor(out=prod[:], in0=in0, in1=in1, op=mult)
                    pf = prod[:].rearrange("p n g h w -> p (n g h w)")
                    for b in range(NBANK):
                        nc.tensor.matmul(
                            out=acc_ps[:, 512 * b : 512 * (b + 1)],
                            lhsT=ident[:],
                            rhs=pf[:, 512 * b : 512 * (b + 1)],
                            start=(kk == 0),
                            stop=(kk == KK - 1),
                        )
                # evict PSUM -> SBUF (fp32) in quarters, alternating DVE (fast
                # 2x fp32 copy, idle once the multiplies are done) and ACT, so
                # stores can start while later quarters are still copying
                av = acc[:].rearrange("p n g h w -> p (n g h w)")
                q = FLAT // 4
                for i in range(4):
                    eng = nc.vector.tensor_copy if i % 2 == 0 else nc.scalar.copy
                    eng(out=av[:, i * q : (i + 1) * q], in_=acc_ps[:, i * q : (i + 1) * q])
            else:
                # both images processed by each instruction (n on a free axis)
                shp = [128, NPC, G, HB, W]
                prev_row = None
                treeacc = tpool.tile(shp, cdt, tag="treeacc")
                for kh in range(K):
                    rowacc = rpool.tile(shp, cdt)
                    for kw in range(K):
                        kk = kh * K + kw
                        if kw % 2 == 0:
                            in0 = in_pad[:, :, :, kh : kh + HB, kw : kw + W]
                        else:
                            in0 = in_pad1[:, :, :, kh : kh + HB, kw - 1 : kw - 1 + W]
                        in1 = w_t[:, :, kk].unsqueeze(2).broadcast_to(shp)
                        if kw == 0:
                            nc.vector.tensor_tensor(
                                out=rowacc[:], in0=in0, in1=in1, op=mult
                            )
                        else:
                            prod = ppool.tile(shp, cdt)
                            nc.vector.tensor_tensor(
                                out=prod[:], in0=in0, in1=in1, op=mult
                            )
                            nc.vector.tensor_tensor(
                                out=rowacc[:], in0=rowacc[:], in1=prod[:], op=add
                            )
                    # pairwise fp16 combine of row sums (keeps the fp16 2x
                    # rate; only the very last add writes fp32)
                    if kh == K - 1:
                        # split by g-half so the first half's output stores can
                        # overlap the second half's add
                        half = G // 2
                        nc.vector.tensor_tensor(
                            out=acc[:, :, 0:half],
                            in0=treeacc[:, :, 0:half],
                            in1=rowacc[:, :, 0:half],
                            op=add,
                        )
                        nc.vector.tensor_tensor(
                            out=acc[:, :, half:G],
                            in0=treeacc[:, :, half:G],
                            in1=rowacc[:, :, half:G],
                            op=add,
                        )
                    elif kh % 2 == 1:
                        if kh == 1:
                            nc.vector.tensor_tensor(
                                out=treeacc[:], in0=prev_row[:], in1=rowacc[:], op=add
                            )
                        else:
                            pair = tpool.tile(shp, cdt, tag="pair")
                            nc.vector.tensor_tensor(
                                out=pair[:], in0=prev_row[:], in1=rowacc[:], op=add
                            )
                            nc.vector.tensor_tensor(
                                out=treeacc[:], in0=treeacc[:], in1=pair[:], op=add
                            )
                        prev_row = None
                    else:
                        prev_row = rowacc
            for g in range(G):
                for n in range(NPC):
                    dsty = y[n].rearrange(
                        "(g cc) (blk hb) w -> g cc blk (hb w)", g=G, blk=BLK
                    )
                    nc.sync.dma_start(
                        out=dsty[g],
                        in_=acc[:, n, g].rearrange("p hb w -> p (hb w)"),
                    )

    nc.compile()
    return nc


def _get_nc(mode=None):
    mode = mode or MODE
    if mode not in _cache:
        _cache[mode] = _build(mode)
    return _cache[mode]


def kernel(input_, weight, _trace=False, _mode=None):
    from concourse.bass_utils import run_bass_kernel_spmd

    mode = _mode or MODE
    nc = _get_nc(mode)
    input_ = np.ascontiguousarray(input_, dtype=np.float32)
    weight = np.ascontiguousarray(weight, dtype=np.float32)
    eye = np.eye(128, dtype=np.float16)
    in_maps = [
        {
            "input": input_[i * NPC : (i + 1) * NPC],
            "weight": weight[i * NPC : (i + 1) * NPC],
            **({"identity": eye} if mode == "fp16pe" else {}),
        }
        for i in range(NCORES)
    ]
    res = run_bass_kernel_spmd(nc, in_maps, list(range(NCORES)), trace=_trace)
    _cache["last_result"] = res
    out = np.concatenate([res.results[i]["output"] for i in range(NCORES)], axis=0)
    return out
